# revision 4
# baseline (speedup 1.0000x reference)
"""Bass/Tile TRN2 kernel for nn_Attn: out = softmax_s(hidden . (W @ enc + b)).

Math: energies[b,s] = hidden[b] . (W enc[s,b] + bias) = (hidden[b] W) . enc[s,b] + const(b).
The const(b) term (hidden.bias) is constant across s, so it cancels in the
softmax exactly; with the spec's attn_b = zeros it is exactly zero anyway.
So per batch element b we need only:
    v_b = hidden[b] @ W                  (tiny [1,H]x[H,H] GEMM, on TensorE)
    E[s] = enc[s, b, :] . v_b            (memory-bound, fused mul+reduce on VectorE)
    out[b, 0, :] = softmax_s(E)          (core-local: max/exp/sum/scale)

Sharding: data-parallel over batch. B == 8 == n_cores, so core b owns batch b,
streams its enc[:, b, :] slice (16.75 MB), and does a fully local softmax.
No collectives.

Layout: s = p*32 + t  (partition p in [0,128), column t in [0,32)) so the final
[128, 32] tile DMAs to the contiguous [4096] output with no transpose.
"""

import numpy as np

import concourse.bass as bass
import concourse.mybir as mybir
import concourse.tile as tile
from concourse import bacc
from concourse.bass_isa import ReduceOp
from concourse.bass_utils import run_bass_kernel_spmd

S, B, H = 4096, 8, 1024
P = 128
NCORES = 8
SCH = S // P          # 32 energy columns per partition
TS = 4                # s-columns per enc DMA tile (tile = [128, 4, 1024] = 2 MiB)
OBLK = H // P         # 8 contraction blocks for v = hid @ W
NHALF = 512           # matmul free-dim limit (one PSUM bank)

_cached_nc = None


def _build():
    nc = bacc.Bacc(
        "TRN2", target_bir_lowering=False, debug=False, num_devices=NCORES
    )
    enc_d = nc.dram_tensor("enc", [S, H], mybir.dt.float32, kind="ExternalInput")
    hid_d = nc.dram_tensor("hid", [H], mybir.dt.float32, kind="ExternalInput")
    w_d = nc.dram_tensor("w", [H, H], mybir.dt.float32, kind="ExternalInput")
    out_d = nc.dram_tensor("out", [S], mybir.dt.float32, kind="ExternalOutput")

    enc_r = enc_d.ap().rearrange("(p q) h -> p q h", p=P)   # [128, 32, 1024]
    hid_r = hid_d.ap().rearrange("(j p) -> p j", p=P)       # [128, 8]
    out_r = out_d.ap().rearrange("(p q) -> p q", p=P)       # [128, 32]

    f32 = mybir.dt.float32
    with tile.TileContext(nc) as tc:
        with (
            tc.tile_pool(name="wpool", bufs=1) as wpool,
            tc.tile_pool(name="encp", bufs=3) as encp,
            tc.tile_pool(name="small", bufs=1) as small,
            tc.tile_pool(name="psum", bufs=1, space=bass.MemorySpace.PSUM) as psum,
        ):
            # ---- prologue: v = hid @ W, then replicate across partitions
            hidT = small.tile([P, OBLK], f32)
            nc.sync.dma_start(hidT[:], hid_r)

            w_tiles = []
            for j in range(OBLK):
                w_t = wpool.tile([P, H], f32, tag=f"w{j}")
                nc.sync.dma_start(w_t[:], w_d.ap()[j * P : (j + 1) * P, :])
                w_tiles.append(w_t)

            v_ps = psum.tile([1, H], f32)
            for half in range(2):
                sl = slice(half * NHALF, (half + 1) * NHALF)
                for j in range(OBLK):
                    nc.tensor.matmul(
                        v_ps[0:1, sl],
                        hidT[:, j : j + 1],
                        w_tiles[j][:, sl],
                        start=(j == 0),
                        stop=(j == OBLK - 1),
                    )
            v_sb = small.tile([1, H], f32)
            nc.vector.tensor_copy(v_sb[:], v_ps[:])
            # v replicated across partitions AND TS-times along free dim, so
            # the per-tile multiply is a single [P, TS, H] tensor_tensor.
            v_rep4 = small.tile([P, TS, H], f32)
            for k in range(TS):
                nc.gpsimd.partition_broadcast(v_rep4[:, k, :], v_sb[:])

            # ---- main: E[p, t] = enc[s=p*32+t, :] . v  (mul on DVE, 3D reduce)
            E = small.tile([P, SCH], f32)
            for t0 in range(0, SCH, TS):
                enc_t = encp.tile([P, TS, H], f32)
                nc.sync.dma_start(enc_t[:], enc_r[:, t0 : t0 + TS, :])
                prod = encp.tile([P, TS, H], f32, tag="prod")
                nc.vector.tensor_mul(prod[:], enc_t[:], v_rep4[:])
                nc.vector.reduce_sum(
                    E[:, t0 : t0 + TS], prod[:], axis=mybir.AxisListType.X
                )

            # ---- softmax over all 4096 energies (core-local)
            m1 = small.tile([P, 1], f32)
            nc.vector.reduce_max(m1[:], E[:], axis=mybir.AxisListType.X)
            nc.gpsimd.partition_all_reduce(m1[:], m1[:], P, ReduceOp.max)
            negm = small.tile([P, 1], f32)
            nc.scalar.mul(negm[:], m1[:], -1.0)
            expt = small.tile([P, SCH], f32)
            sums = small.tile([P, 1], f32)
            nc.scalar.activation(
                expt[:],
                E[:],
                mybir.ActivationFunctionType.Exp,
                bias=negm[:],
                accum_out=sums[:],
            )
            nc.gpsimd.partition_all_reduce(sums[:], sums[:], P, ReduceOp.add)
            rs = small.tile([P, 1], f32)
            nc.vector.reciprocal(rs[:], sums[:])
            outt = small.tile([P, SCH], f32)
            nc.vector.tensor_scalar_mul(outt[:], expt[:], rs[:])
            nc.sync.dma_start(out_r, outt[:])

    nc.compile()
    return nc


def _get_nc():
    global _cached_nc
    if _cached_nc is None:
        _cached_nc = _build()
    return _cached_nc


def run(inputs, trace=False):
    """Shard, run SPMD on 8 cores, gather. Returns (output, BassKernelResults)."""
    nc = _get_nc()
    hidden = np.ascontiguousarray(np.asarray(inputs["hidden"], dtype=np.float32))
    enc = np.asarray(inputs["encoder_outputs"], dtype=np.float32)
    w = np.ascontiguousarray(np.asarray(inputs["attn_w"], dtype=np.float32))
    # attn_b is a constant shift across s per batch -> cancels in softmax.

    in_maps = []
    for b in range(NCORES):
        in_maps.append(
            {
                "enc": np.ascontiguousarray(enc[:, b, :]),
                "hid": np.ascontiguousarray(hidden[0, b, :]),
                "w": w,
            }
        )
    res = run_bass_kernel_spmd(
        nc, in_maps, core_ids=list(range(NCORES)), trace=trace
    )
    out = np.stack([res.results[b]["out"] for b in range(NCORES)], axis=0)
    return out[:, None, :].astype(np.float32), res


def kernel(hidden, encoder_outputs, attn_w, attn_b=None, **_unused):
    out, _ = run(
        {
            "hidden": hidden,
            "encoder_outputs": encoder_outputs,
            "attn_w": attn_w,
        }
    )
    return out


# revision 8
# speedup vs baseline: 35.0614x; 35.0614x over previous
"""Bass/Tile TRN2 kernel for nn_Attn: out = softmax_s(hidden . (W @ enc + b)).

Math: energies[b,s] = hidden[b] . (W enc[s,b] + bias) = (hidden[b] W) . enc[s,b] + const(b).
The const(b) term (hidden.bias) is constant across s, so it cancels in the
softmax exactly; with the spec's attn_b = zeros it is exactly zero anyway.
So per batch element b we need only:
    v_b = hidden[b] @ W                  (tiny [1,H]x[H,H] GEMM, on TensorE)
    E[s] = enc[s, b, :] . v_b            (memory-bound fused mul+reduce on VectorE)
    out[b, 0, :] = softmax_s(E)          (core-local: max/exp/sum/scale)

Sharding: data-parallel over batch. B == 8 == n_cores, so core b owns batch b,
streams its enc[:, b, :] slice (16.75 MB), and does a fully local softmax.
No collectives.

Layout: s = p*32 + t  (partition p in [0,128), column t in [0,32)) so the final
[128, 32] tile DMAs to the contiguous [4096] output with no transpose.
The per-(s-tile) dot is one scalar_tensor_tensor per 128 s-rows:
    res = (enc_slice * 1.0) * v_rep ; E[:, col] = sum_h res   (fused accum)
"""

import numpy as np

import concourse.bass as bass
import concourse.mybir as mybir
import concourse.tile as tile
from concourse import bacc
from concourse.bass_isa import ReduceOp
from concourse.bass_utils import run_bass_kernel_spmd

S, B, H = 4096, 8, 1024
P = 128
NCORES = 8
SCH = S // P          # 32 energy columns per partition
TS = 4                # s-columns per enc DMA tile (tile = [128, 4, 1024] = 2 MiB)
OBLK = H // P         # 8 contraction blocks for v = hid @ W
NHALF = 512           # matmul free-dim limit (one PSUM bank)

_cached_nc = None


def _build():
    nc = bacc.Bacc(
        "TRN2", target_bir_lowering=False, debug=False, num_devices=NCORES
    )
    enc_d = nc.dram_tensor("enc", [S, H], mybir.dt.float32, kind="ExternalInput")
    # hidT is the per-core hidden vector pre-transposed on host to [128, 8]:
    # hidT[p, j] = hidden[j*128 + p], so it DMAs contiguously and is directly
    # the matmul lhsT ([K=o-block, M=1] columns).
    hid_d = nc.dram_tensor("hidT", [P, OBLK], mybir.dt.float32, kind="ExternalInput")
    w_d = nc.dram_tensor("w", [H, H], mybir.dt.float32, kind="ExternalInput")
    out_d = nc.dram_tensor("out", [S], mybir.dt.float32, kind="ExternalOutput")

    enc_r = enc_d.ap().rearrange("(p q) h -> p q h", p=P)   # [128, 32, 1024]
    out_r = out_d.ap().rearrange("(p q) -> p q", p=P)       # [128, 32]

    f32 = mybir.dt.float32
    with tile.TileContext(nc) as tc:
        with (
            tc.tile_pool(name="wpool", bufs=1) as wpool,
            tc.tile_pool(name="encp", bufs=5) as encp,
            tc.tile_pool(name="small", bufs=1) as small,
            tc.tile_pool(name="psum", bufs=1, space=bass.MemorySpace.PSUM) as psum,
        ):
            # ---- PE warmup: ~5us of junk matmuls so the HAM clock-gate lifts
            # (cold PE runs at 1.2 GHz; the v GEMM then runs at full rate).
            wu = small.tile([P, NHALF], f32)
            nc.vector.memset(wu[:], 1.0)
            wu_ps = psum.tile([1, NHALF], f32)
            for i in range(3):
                nc.tensor.matmul(
                    wu_ps[0:1, :], wu[:, 0:1], wu[:],
                    start=(i == 0), stop=(i == 2),
                )

            # ---- prologue: v = hid @ W on PE, then replicate across partitions
            hidT = small.tile([P, OBLK], f32)
            nc.sync.dma_start(hidT[:], hid_d.ap())

            w_tiles = []
            for j in range(OBLK):
                w_t = wpool.tile([P, H], f32, tag=f"w{j}", name=f"w{j}")
                nc.sync.dma_start(w_t[:], w_d.ap()[j * P : (j + 1) * P, :])
                w_tiles.append(w_t)

            v_ps = psum.tile([1, H], f32)
            for half in range(2):
                sl = slice(half * NHALF, (half + 1) * NHALF)
                for j in range(OBLK):
                    nc.tensor.matmul(
                        v_ps[0:1, sl],
                        hidT[:, j : j + 1],
                        w_tiles[j][:, sl],
                        start=(j == 0),
                        stop=(j == OBLK - 1),
                    )
            v_sb = small.tile([1, H], f32)
            nc.vector.tensor_copy(v_sb[:], v_ps[:])
            v_rep = small.tile([P, H], f32)
            nc.gpsimd.partition_broadcast(v_rep[:], v_sb[:])

            # ---- main: E[p, t] = enc[s=p*32+t, :] . v  (fused mul + accum)
            E = small.tile([P, SCH], f32)
            scratch = small.tile([P, H], f32)
            for t0 in range(0, SCH, TS):
                enc_t = encp.tile([P, TS, H], f32)
                nc.sync.dma_start(enc_t[:], enc_r[:, t0 : t0 + TS, :])
                for k in range(TS):
                    nc.vector.scalar_tensor_tensor(
                        scratch[:],
                        enc_t[:, k, :],
                        1.0,
                        v_rep[:],
                        op0=mybir.AluOpType.mult,
                        op1=mybir.AluOpType.mult,
                        accum_out=E[:, t0 + k : t0 + k + 1],
                    )

            # ---- softmax over all 4096 energies (core-local)
            m1 = small.tile([P, 1], f32)
            nc.vector.reduce_max(m1[:], E[:], axis=mybir.AxisListType.X)
            nc.gpsimd.partition_all_reduce(m1[:], m1[:], P, ReduceOp.max)
            negm = small.tile([P, 1], f32)
            nc.scalar.mul(negm[:], m1[:], -1.0)
            expt = small.tile([P, SCH], f32)
            sums = small.tile([P, 1], f32)
            nc.scalar.activation(
                expt[:],
                E[:],
                mybir.ActivationFunctionType.Exp,
                bias=negm[:],
                accum_out=sums[:],
            )
            nc.gpsimd.partition_all_reduce(sums[:], sums[:], P, ReduceOp.add)
            rs = small.tile([P, 1], f32)
            nc.vector.reciprocal(rs[:], sums[:])
            outt = small.tile([P, SCH], f32)
            nc.vector.tensor_scalar_mul(outt[:], expt[:], rs[:])
            nc.sync.dma_start(out_r, outt[:])

    nc.compile()
    return nc


def _get_nc():
    global _cached_nc
    if _cached_nc is None:
        _cached_nc = _build()
    return _cached_nc


def shard_inputs(inputs):
    """Per-core input maps: core b gets batch b's enc slice and hidden
    (pre-transposed to the matmul lhsT layout); W is replicated."""
    hidden = np.ascontiguousarray(np.asarray(inputs["hidden"], dtype=np.float32))
    enc = np.asarray(inputs["encoder_outputs"], dtype=np.float32)
    w = np.ascontiguousarray(np.asarray(inputs["attn_w"], dtype=np.float32))
    # attn_b is a constant shift across s per batch -> cancels in softmax.
    in_maps = []
    for b in range(NCORES):
        in_maps.append(
            {
                "enc": np.ascontiguousarray(enc[:, b, :]),
                "hidT": np.ascontiguousarray(
                    hidden[0, b, :].reshape(OBLK, P).T
                ),
                "w": w,
            }
        )
    return in_maps


def run(inputs, trace=False):
    """Shard, run SPMD on 8 cores, gather. Returns (output, BassKernelResults)."""
    nc = _get_nc()
    in_maps = shard_inputs(inputs)
    res = run_bass_kernel_spmd(
        nc, in_maps, core_ids=list(range(NCORES)), trace=trace
    )
    out = np.stack([res.results[b]["out"] for b in range(NCORES)], axis=0)
    return out[:, None, :].astype(np.float32), res


def kernel(hidden, encoder_outputs, attn_w, attn_b=None, **_unused):
    out, _ = run(
        {
            "hidden": hidden,
            "encoder_outputs": encoder_outputs,
            "attn_w": attn_w,
        }
    )
    return out


# revision 13
# speedup vs baseline: 36.9737x; 1.0545x over previous
"""Bass/Tile TRN2 kernel for nn_Attn: out = softmax_s(hidden . (W @ enc + b)).

Math: energies[b,s] = hidden[b] . (W enc[s,b] + bias) = (hidden[b] W) . enc[s,b] + const(b).
The const(b) term (hidden.bias) is constant across s, so it cancels in the
softmax exactly; with the spec's attn_b = zeros it is exactly zero anyway.
So per batch element b we need only:
    v_b = hidden[b] @ W                  (tiny [1,H]x[H,H] GEMM, on TensorE)
    E[s] = enc[s, b, :] . v_b            (memory-bound fused mul+reduce on VectorE)
    out[b, 0, :] = softmax_s(E)          (core-local: max/exp/sum/scale)

Sharding: data-parallel over batch. B == 8 == n_cores, so core b owns batch b,
streams its enc[:, b, :] slice (16.75 MB), and does a fully local softmax.
No collectives.

Layout: s = p*32 + t  (partition p in [0,128), column t in [0,32)) so the final
[128, 32] tile DMAs to the contiguous [4096] output with no transpose.
The per-(s-tile) dot is one scalar_tensor_tensor per 128 s-rows:
    res = (enc_slice * 1.0) * v_rep ; E[:, col] = sum_h res   (fused accum)
"""

import numpy as np

import concourse.bass as bass
import concourse.mybir as mybir
import concourse.tile as tile
from concourse import bacc
from concourse.bass_isa import ReduceOp
from concourse.bass_utils import run_bass_kernel_spmd

S, B, H = 4096, 8, 1024
P = 128
NCORES = 8
SCH = S // P          # 32 energy columns per partition
TS = 4                # s-columns per enc DMA tile (tile = [128, 4, 1024] = 2 MiB)
OBLK = H // P         # 8 contraction blocks for v = hid @ W
NHALF = 512           # matmul free-dim limit (one PSUM bank)

_cached_nc = None


def _build():
    nc = bacc.Bacc(
        "TRN2", target_bir_lowering=False, debug=False, num_devices=NCORES
    )
    enc_d = nc.dram_tensor("enc", [S, H], mybir.dt.float32, kind="ExternalInput")
    # hidT is the per-core hidden vector pre-transposed on host to [128, 8]:
    # hidT[p, j] = hidden[j*128 + p], so it DMAs contiguously and is directly
    # the matmul lhsT ([K=o-block, M=1] columns).
    hid_d = nc.dram_tensor("hidT", [P, OBLK], mybir.dt.float32, kind="ExternalInput")
    w_d = nc.dram_tensor("w", [H, H], mybir.dt.float32, kind="ExternalInput")
    out_d = nc.dram_tensor("out", [S], mybir.dt.float32, kind="ExternalOutput")

    enc_r = enc_d.ap().rearrange("(p q) h -> p q h", p=P)   # [128, 32, 1024]
    out_r = out_d.ap().rearrange("(p q) -> p q", p=P)       # [128, 32]

    f32 = mybir.dt.float32
    with tile.TileContext(nc) as tc:
        with (
            tc.tile_pool(name="wpool", bufs=1) as wpool,
            tc.tile_pool(name="encp", bufs=5) as encp,
            tc.tile_pool(name="small", bufs=1) as small,
            tc.tile_pool(name="psum", bufs=1, space=bass.MemorySpace.PSUM) as psum,
        ):
            # ---- PE warmup: junk matmuls sized to end as W[0] lands, so the
            # HAM clock-gate lifts and the v GEMM runs at full (warm) rate.
            wu = small.tile([P, NHALF], f32)
            nc.vector.memset(wu[:], 1.0)
            wu_ps = psum.tile([1, NHALF], f32)
            NWU = 8
            for i in range(NWU):
                nc.tensor.matmul(
                    wu_ps[0:1, 0:128], wu[:, 0:1], wu[:, 0:128],
                    start=(i == 0), stop=(i == NWU - 1),
                )

            # ---- prologue: v = hid @ W on PE, then replicate across partitions
            hidT = small.tile([P, OBLK], f32)
            nc.sync.dma_start(hidT[:], hid_d.ap())
            w_tiles = []
            for j in range(OBLK):
                w_t = wpool.tile([P, H], f32, tag=f"w{j}", name=f"w{j}")
                nc.sync.dma_start(w_t[:], w_d.ap()[j * P : (j + 1) * P, :])
                w_tiles.append(w_t)

            # j-outer order: matmuls chase the W-tile DMAs, so the GEMM ends
            # ~2 matmuls after the last W byte instead of queueing all of
            # half-1 behind half-0.
            v_ps = psum.tile([1, H], f32)
            for j in range(OBLK):
                for half in range(2):
                    sl = slice(half * NHALF, (half + 1) * NHALF)
                    nc.tensor.matmul(
                        v_ps[0:1, sl],
                        hidT[:, j : j + 1],
                        w_tiles[j][:, sl],
                        start=(j == 0),
                        stop=(j == OBLK - 1),
                    )
            v_sb = small.tile([1, H], f32)
            nc.vector.tensor_copy(v_sb[:], v_ps[:])
            v_rep = small.tile([P, H], f32)
            nc.gpsimd.partition_broadcast(v_rep[:], v_sb[:])

            # ---- main: E[p, t] = enc[s=p*32+t, :] . v  (fused mul + accum)
            # One DMA per s-column (512 KiB) so each scalar_tensor_tensor
            # starts as soon as its own column lands — the DVE trails the
            # DMA stream by ~1 op instead of a whole 2 MiB tile.
            E = small.tile([P, SCH], f32)
            scratch = small.tile([P, H], f32)
            for t0 in range(0, SCH, TS):
                enc_t = encp.tile([P, TS, H], f32, name="enc_t")
                for k in range(TS):
                    nc.sync.dma_start(
                        enc_t[:, k, :], enc_r[:, t0 + k, :]
                    )
                for k in range(TS):
                    nc.vector.scalar_tensor_tensor(
                        scratch[:],
                        enc_t[:, k, :],
                        1.0,
                        v_rep[:],
                        op0=mybir.AluOpType.mult,
                        op1=mybir.AluOpType.mult,
                        accum_out=E[:, t0 + k : t0 + k + 1],
                    )

            # ---- softmax over all 4096 energies (core-local)
            m1 = small.tile([P, 1], f32)
            nc.vector.reduce_max(m1[:], E[:], axis=mybir.AxisListType.X)
            nc.gpsimd.partition_all_reduce(m1[:], m1[:], P, ReduceOp.max)
            negm = small.tile([P, 1], f32)
            nc.scalar.mul(negm[:], m1[:], -1.0)
            expt = small.tile([P, SCH], f32)
            sums = small.tile([P, 1], f32)
            nc.scalar.activation(
                expt[:],
                E[:],
                mybir.ActivationFunctionType.Exp,
                bias=negm[:],
                accum_out=sums[:],
            )
            nc.gpsimd.partition_all_reduce(sums[:], sums[:], P, ReduceOp.add)
            rs = small.tile([P, 1], f32)
            nc.vector.reciprocal(rs[:], sums[:])
            outt = small.tile([P, SCH], f32)
            nc.vector.tensor_scalar_mul(outt[:], expt[:], rs[:])
            nc.sync.dma_start(out_r, outt[:])

    nc.compile()
    return nc


def _get_nc():
    global _cached_nc
    if _cached_nc is None:
        _cached_nc = _build()
    return _cached_nc


def shard_inputs(inputs):
    """Per-core input maps: core b gets batch b's enc slice and hidden
    (pre-transposed to the matmul lhsT layout); W is replicated."""
    hidden = np.ascontiguousarray(np.asarray(inputs["hidden"], dtype=np.float32))
    enc = np.asarray(inputs["encoder_outputs"], dtype=np.float32)
    w = np.ascontiguousarray(np.asarray(inputs["attn_w"], dtype=np.float32))
    # attn_b is a constant shift across s per batch -> cancels in softmax.
    in_maps = []
    for b in range(NCORES):
        in_maps.append(
            {
                "enc": np.ascontiguousarray(enc[:, b, :]),
                "hidT": np.ascontiguousarray(
                    hidden[0, b, :].reshape(OBLK, P).T
                ),
                "w": w,
            }
        )
    return in_maps


def run(inputs, trace=False):
    """Shard, run SPMD on 8 cores, gather. Returns (output, BassKernelResults)."""
    nc = _get_nc()
    in_maps = shard_inputs(inputs)
    res = run_bass_kernel_spmd(
        nc, in_maps, core_ids=list(range(NCORES)), trace=trace
    )
    out = np.stack([res.results[b]["out"] for b in range(NCORES)], axis=0)
    return out[:, None, :].astype(np.float32), res


def kernel(hidden, encoder_outputs, attn_w, attn_b=None, **_unused):
    out, _ = run(
        {
            "hidden": hidden,
            "encoder_outputs": encoder_outputs,
            "attn_w": attn_w,
        }
    )
    return out
